# revision 4
# baseline (speedup 1.0000x reference)
"""Trainium2 Bass kernel for AngularSymmetryMod (ANI-style angular symmetry functions).

Math: for each (b,i), sum over atom pairs (j,k) of
    (1 + lam*cos(theta - theta_t))^4 * exp(-ita*((R_ij+R_ik)/2 - Rs)^2) * f_ij*f_ik * 2^(1-zeta)
over a 40-point parameter grid (lam in {+-1}, 5 Rs values, 4 theta_t values).

Key factorization: (1+lam*cos(phi))^4 = 35/8 + 7lam*cos(phi) + 3.5cos(2phi) + lam*cos(3phi)
+ cos(4phi)/8 with phi = theta - theta_t, so every l is a fixed linear combination of 9
harmonics [1, cos(k*theta), sin(k*theta)]_{k<=4} times one of 5 radial gaussians. The kernel
reduces 45 = 5x9 basis sums S[b,i,r,h] over (j,k) and applies the constant 40x45 matrix on-chip.

Sharding: data-parallel over batch (16 molecules -> 2 per core on 8 cores). No collectives.
Layout per core: 128 partitions = (b_loc:2, i:32, jhalf:2), free = (jB:16, k:32).
"""

import sys
import numpy as np

sys.path.insert(0, "/opt/trn_rl_repo")

from contextlib import ExitStack

import concourse.bass as bass
import concourse.tile as tile
from concourse import bacc, mybir
from concourse.bass_utils import run_bass_kernel_spmd

B, N, L = 16, 32, 40
NCORES = 8
B_LOC = B // NCORES  # 2
P = 128  # partitions = B_LOC * N * 2
NJB, NK = 16, 32  # free layout

BOHR = 0.52917721092
ITA = 1.12
ZETA = 4.0
RS_VALS = np.array([0.5, 1.17, 1.83, 2.5, 3.17]) / BOHR
THETAS = np.array([0.0, 1.57, 3.14, 4.71])
LAMBDAS = np.array([1.0, -1.0])
NR, NH = 5, 9
NRH = NR * NH  # 45

F32 = mybir.dt.float32
I32 = mybir.dt.int32
OP = mybir.AluOpType
ACT = mybir.ActivationFunctionType

# input packing offsets along the free axis of the single [128, 243] input tensor
OFF_CI = 0      # [3]   coords of atom i (per-partition scalars)
OFF_CJ = 3      # [48]  coords of the 16 j-atoms of this partition's half, layout [c][jB]
OFF_CK = 51     # [96]  coords of all 32 k-atoms, layout [c][k]
OFF_UJ = 147    # [16]  d[b,i,j]
OFF_UK = 163    # [32]  d[b,i,k]
OFF_FJ = 195    # [16]  d_cutoff[b,i,j]
OFF_FK = 211    # [32]  d_cutoff[b,i,k]
NIN = 243


def _a2_matrix():
    """A2[l, r*9+h]: out[b,i,l] = sum_rh A2[l,rh] * S[b,i,r,h]."""
    Lm, Im, Zm, Rm, Tm = np.meshgrid(LAMBDAS, [ITA], [ZETA], RS_VALS, THETAS)
    lam_f = Lm.flatten()
    rs_f = Rm.flatten()
    tt_f = Tm.flatten()
    A2 = np.zeros((L, NRH), dtype=np.float64)
    for l in range(L):
        lam, tt = lam_f[l], tt_f[l]
        r = int(np.argmin(np.abs(RS_VALS - rs_f[l])))
        Ah = [
            35.0 / 8.0,
            7.0 * lam * np.cos(tt), 7.0 * lam * np.sin(tt),
            3.5 * np.cos(2 * tt), 3.5 * np.sin(2 * tt),
            lam * np.cos(3 * tt), lam * np.sin(3 * tt),
            0.125 * np.cos(4 * tt), 0.125 * np.sin(4 * tt),
        ]
        for h in range(NH):
            A2[l, r * NH + h] = Ah[h] * 2.0 ** (1.0 - ZETA)
    return A2.astype(np.float32)


def _build():
    nc = bacc.Bacc("TRN2", target_bir_lowering=False, debug=False)
    inp_d = nc.declare_dram_parameter("inp", [P, NIN], F32, isOutput=False)
    cst_d = nc.declare_dram_parameter("cst", [P, 168], F32, isOutput=False)
    out_d = nc.declare_dram_parameter("out", [B_LOC * N, L], F32, isOutput=True)

    TWO_PI = float(2.0 * np.pi)
    INV_2PI = float(1.0 / (2.0 * np.pi))

    with tile.TileContext(nc) as tc, ExitStack() as ctx:
        pool = ctx.enter_context(tc.tile_pool(name="sb", bufs=1))
        scr_pool = ctx.enter_context(tc.tile_pool(name="scr", bufs=2))
        rad_pool = ctx.enter_context(tc.tile_pool(name="rad", bufs=2))
        w_pool = ctx.enter_context(tc.tile_pool(name="w", bufs=2))
        psum = ctx.enter_context(tc.tile_pool(name="ps", bufs=1, space="PSUM"))

        inp = pool.tile([P, NIN], F32, name="inp", tag="inp")
        cst = pool.tile([P, 168], F32, name="cst", tag="cst")
        nc.sync.dma_start(inp[:], inp_d[:])
        nc.sync.dma_start(cst[:], cst_d[:])

        def big(tag):
            return pool.tile([P, NJB, NK], F32, name=tag, tag=tag)

        # ---- small prep: vj = xj - xi (= -v_j), vk = xk - xi (= -v_k); dot uses vj.vk ----
        vj = pool.tile([P, 3, NJB], F32, name="vj", tag="vj")
        vk = pool.tile([P, 3, NK], F32, name="vk", tag="vk")
        for c in range(3):
            ci_c = inp[:, OFF_CI + c : OFF_CI + c + 1]
            nc.vector.tensor_scalar(
                vj[:, c, :], inp[:, OFF_CJ + c * NJB : OFF_CJ + (c + 1) * NJB],
                ci_c, None, OP.subtract)
            nc.vector.tensor_scalar(
                vk[:, c, :], inp[:, OFF_CK + c * NK : OFF_CK + (c + 1) * NK],
                ci_c, None, OP.subtract)

        def jview(ap2d):  # [P, 16] -> [P, 16, 32] broadcast along k
            return ap2d.unsqueeze(2).to_broadcast([P, NJB, NK])

        def kview(ap2d):  # [P, 32] -> [P, 16, 32] broadcast along jB
            return ap2d.unsqueeze(1).to_broadcast([P, NJB, NK])

        # ---- dot = sum_c vj_c * vk_c ----
        prod = big("prod")
        dot = big("dot")
        nc.vector.tensor_tensor(dot[:], jview(vj[:, 0, :]), kview(vk[:, 0, :]), OP.mult)
        for c in (1, 2):
            nc.vector.tensor_tensor(prod[:], jview(vj[:, c, :]), kview(vk[:, c, :]), OP.mult)
            nc.vector.tensor_tensor(dot[:], dot[:], prod[:], OP.add)

        # ---- thp = theta / 2pi  (theta = dot / (uj*uk + 1e-5)) ----
        ujv = jview(inp[:, OFF_UJ : OFF_UJ + NJB])
        ukv = kview(inp[:, OFF_UK : OFF_UK + NK])
        den = big("den")
        nc.vector.tensor_tensor(den[:], ujv, ukv, OP.mult)
        nc.vector.tensor_scalar(den[:], den[:], 1e-5, TWO_PI, OP.add, OP.mult)
        rden = big("rden")
        nc.vector.reciprocal(rden[:], den[:])
        thp = big("thp")
        nc.vector.tensor_tensor(thp[:], dot[:], rden[:], OP.mult)

        # ---- range-reduce + sin/cos via Sin table (round-to-nearest int32 convert) ----
        def trig(src_ap, shift, out_tag):
            r = big(out_tag + "_r")
            if shift == 0.0:
                r = src_ap
            else:
                nc.vector.tensor_scalar(r[:], src_ap[:], shift, None, OP.add)
            n_i = pool.tile([P, NJB, NK], I32, name=out_tag + "_n", tag=out_tag + "_n")
            nc.vector.tensor_copy(n_i[:], r[:])
            n_f = big(out_tag + "_nf")
            nc.vector.tensor_copy(n_f[:], n_i[:])
            fr = big(out_tag + "_fr")
            nc.vector.tensor_tensor(fr[:], r[:], n_f[:], OP.subtract)
            o = big(out_tag)
            nc.scalar.activation(o[:], fr[:], ACT.Sin, scale=TWO_PI)
            return o

        s1 = trig(thp, 0.0, "s1")
        c1 = trig(thp, 0.25, "c1")

        # ---- harmonics via Chebyshev-style recurrences ----
        c2 = big("c2")
        nc.vector.tensor_tensor(c2[:], c1[:], c1[:], OP.mult)
        nc.vector.tensor_scalar(c2[:], c2[:], 2.0, 1.0, OP.mult, OP.subtract)
        s2 = big("s2")
        nc.vector.scalar_tensor_tensor(s2[:], s1[:], 2.0, c1[:], OP.mult, OP.mult)
        c3 = big("c3")
        nc.vector.scalar_tensor_tensor(c3[:], c2[:], 2.0, c1[:], OP.mult, OP.mult)
        nc.vector.tensor_tensor(c3[:], c3[:], c1[:], OP.subtract)
        s3 = big("s3")
        nc.vector.scalar_tensor_tensor(s3[:], c2[:], 2.0, s1[:], OP.mult, OP.mult)
        nc.vector.tensor_tensor(s3[:], s3[:], s1[:], OP.add)
        c4 = big("c4")
        nc.vector.tensor_tensor(c4[:], c2[:], c2[:], OP.mult)
        nc.vector.tensor_scalar(c4[:], c4[:], 2.0, 1.0, OP.mult, OP.subtract)
        s4 = big("s4")
        nc.vector.scalar_tensor_tensor(s4[:], s2[:], 2.0, c2[:], OP.mult, OP.mult)
        H = [None, c1, s1, c2, s2, c3, s3, c4, s4]  # h=0 is implicit ones

        # ---- radial prep: q = uj + uk; cut = fj * fk ----
        q = big("q")
        nc.vector.tensor_tensor(q[:], ujv, ukv, OP.add)
        cut = big("cut")
        nc.vector.tensor_tensor(
            cut[:], jview(inp[:, OFF_FJ : OFF_FJ + NJB]), kview(inp[:, OFF_FK : OFF_FK + NK]),
            OP.mult)

        rs_bias = pool.tile([P, NR], F32, name="rsb", tag="rsb")
        for r in range(NR):
            nc.vector.memset(rs_bias[:, r : r + 1], float(-RS_VALS[r]))

        spart = pool.tile([P, 48], F32, name="spart", tag="spart")

        # ---- per radial r: rad = exp(-ita*(q/2 - Rs)^2); W = cut*rad; 9 fused reduce ----
        for r in range(NR):
            sq = rad_pool.tile([P, NJB, NK], F32, name="sq", tag="sq")
            nc.scalar.activation(sq[:], q[:], ACT.Square, bias=rs_bias[:, r : r + 1], scale=0.5)
            rad = rad_pool.tile([P, NJB, NK], F32, name="rad", tag="rad")
            nc.scalar.activation(rad[:], sq[:], ACT.Exp, scale=float(-ITA))
            W = w_pool.tile([P, NJB, NK], F32, name="w", tag="w")
            nc.vector.tensor_tensor(W[:], cut[:], rad[:], OP.mult)
            nc.vector.tensor_reduce(spart[:, r * NH : r * NH + 1], W[:], mybir.AxisListType.XY, OP.add)
            for h in range(1, NH):
                scr = scr_pool.tile([P, NJB, NK], F32, name="scr", tag="scr")
                nc.vector.scalar_tensor_tensor(
                    scr[:], W[:], 0.0, H[h][:], OP.bypass, OP.mult,
                    accum_out=spart[:, r * NH + h : r * NH + h + 1])

        # ---- combine: S2[bi,rh] = pairsum; out[bi,l] = S2 @ A2^T ----
        mpair = cst[:, 0:64]
        ident = cst[0:64, 64:128]
        a2t = cst[0:45, 128:168]
        s2p = psum.tile([64, NRH], F32, name="s2p", tag="s2p")
        nc.tensor.matmul(s2p[:], mpair, spart[:, 0:NRH])
        s2s = pool.tile([64, NRH], F32, name="s2s", tag="s2s")
        nc.vector.tensor_copy(s2s[:], s2p[:])
        s2tp = psum.tile([NRH, 64], F32, name="s2tp", tag="s2tp")
        nc.tensor.transpose(s2tp[:], s2s[:], ident)
        s2ts = pool.tile([NRH, 64], F32, name="s2ts", tag="s2ts")
        nc.vector.tensor_copy(s2ts[:], s2tp[:])
        outp = psum.tile([64, L], F32, name="outp", tag="outp")
        nc.tensor.matmul(outp[:], s2ts[:], a2t)
        outs = pool.tile([64, L], F32, name="outs", tag="outs")
        nc.vector.tensor_copy(outs[:], outp[:])
        nc.sync.dma_start(out_d[:], outs[:])

    nc.compile()
    return nc


def _ensure_ntff_hook():
    """Register the axon NTFF profiling hook if the image lacks antenv.axon_hooks."""
    import types

    try:
        from antenv.axon_hooks import get_axon_ntff_profile_hook
        if get_axon_ntff_profile_hook() is not None:
            return
        have_mod = True
    except ImportError:
        have_mod = False
    try:
        if "/root/.axon_site" not in sys.path:
            sys.path.insert(0, "/root/.axon_site")
        from trn_agent_boot.trn_boot import _ntff_profile_via_ctypes

        hook = _ntff_profile_via_ctypes("/opt/axon/libaxon_pjrt.so")
        if hook is None:
            return
    except Exception:
        return
    if have_mod:
        from antenv import axon_hooks
        axon_hooks.set_axon_ntff_profile_hook(hook)
    else:
        m = types.ModuleType("antenv.axon_hooks")
        _h = [hook]
        m.get_axon_ntff_profile_hook = lambda: _h[0]
        m.set_axon_ntff_profile_hook = lambda h: _h.__setitem__(0, h)
        import antenv
        antenv.axon_hooks = m
        sys.modules["antenv.axon_hooks"] = m


_NC = None


def _get_nc():
    global _NC
    if _NC is None:
        _NC = _build()
    return _NC


def _host_pack(d_cutoff, d, atom_coordinates):
    """Pure gather/replication of raw inputs into the per-core [128, 243] layout."""
    d_cutoff = np.ascontiguousarray(d_cutoff, dtype=np.float32)
    d = np.ascontiguousarray(d, dtype=np.float32)
    coords = np.ascontiguousarray(atom_coordinates, dtype=np.float32)

    p = np.arange(P)
    b_of_p = p // (N * 2)
    i_of_p = (p // 2) % N
    jA = p % 2
    jfull = jA[:, None] * NJB + np.arange(NJB)[None, :]  # [P, 16]
    karr = np.arange(NK)

    in_maps = []
    for c in range(NCORES):
        cd = coords[c * B_LOC : (c + 1) * B_LOC]
        dd = d[c * B_LOC : (c + 1) * B_LOC]
        fc = d_cutoff[c * B_LOC : (c + 1) * B_LOC]
        buf = np.empty((P, NIN), dtype=np.float32)
        buf[:, OFF_CI : OFF_CI + 3] = cd[b_of_p, i_of_p]                     # [P,3]
        cjv = cd[b_of_p[:, None], jfull]                                     # [P,16,3]
        buf[:, OFF_CJ : OFF_CJ + 48] = cjv.transpose(0, 2, 1).reshape(P, 48)
        ckv = cd[b_of_p]                                                     # [P,32,3]
        buf[:, OFF_CK : OFF_CK + 96] = ckv.transpose(0, 2, 1).reshape(P, 96)
        buf[:, OFF_UJ : OFF_UJ + 16] = dd[b_of_p[:, None], i_of_p[:, None], jfull]
        buf[:, OFF_UK : OFF_UK + 32] = dd[b_of_p[:, None], i_of_p[:, None], karr[None, :]]
        buf[:, OFF_FJ : OFF_FJ + 16] = fc[b_of_p[:, None], i_of_p[:, None], jfull]
        buf[:, OFF_FK : OFF_FK + 32] = fc[b_of_p[:, None], i_of_p[:, None], karr[None, :]]
        in_maps.append({"inp": buf, "cst": _const_blob()})
    return in_maps


_CST = None


def _const_blob():
    global _CST
    if _CST is None:
        cst = np.zeros((P, 168), dtype=np.float32)
        cst[:, 0:64] = np.repeat(np.eye(64, dtype=np.float32), 2, axis=0)
        cst[0:64, 64:128] = np.eye(64, dtype=np.float32)
        cst[0:45, 128:168] = _a2_matrix().T
        _CST = cst
    return _CST


def kernel(d_cutoff, d, atom_coordinates, _trace=False):
    if _trace:
        _ensure_ntff_hook()
    nc = _get_nc()
    in_maps = _host_pack(d_cutoff, d, atom_coordinates)
    res = run_bass_kernel_spmd(nc, in_maps, core_ids=list(range(NCORES)), trace=_trace)
    out = np.concatenate(
        [res.results[c]["out"].reshape(B_LOC, N, L) for c in range(NCORES)], axis=0
    ).astype(np.float32)
    if _trace:
        kernel._last_results = res
    return out


# revision 6
# speedup vs baseline: 1.2898x; 1.2898x over previous
"""Trainium2 Bass kernel for AngularSymmetryMod (ANI-style angular symmetry functions).

Math: out[b,i,l] = sum_{j,k} (1+lam*cos(theta-theta_t))^zeta * exp(-ita*((R_ij+R_ik)/2-Rs)^2)
                            * f_ij*f_ik * 2^(1-zeta)
over a 40-point parameter grid l=(lam in {+-1}, 5 Rs values, 4 theta_t values), zeta=4.

Key observations exploited here:
 1. theta_t = {0.0, 1.57, 3.14, 4.71} are (to 8e-4) the exact quadrants {0, pi/2, pi, 3pi/2},
    so cos(theta-theta_t) = {c, s, -c, -s} and (1+lam*cos(theta-theta_t))^4 collapses to just
    FOUR distinct fields: (1+c)^4, (1+s)^4, (1-c)^4, (1-s)^4 — each computed with two chained
    Square activations on the ScalarEngine. (Validated: 2.2e-4 rel err vs the f32 reference.)
 2. Each of the 40 outputs is S[r, m] for one of 5 radial gaussians r and one of the 4 angular
    fields m — 20 distinct reductions; the 40 outputs are a column remap done by the output DMA.
 3. sin/cos need exact-range reduction (theta spans +-2.3e6): theta/2pi - round(theta/2pi) via
    the DVE's round-to-nearest f32->int32 convert, then the Sin table on [-pi, pi].

Sharding: data-parallel over batch (16 molecules -> 2 per core on 8 cores). No collectives.
Layout per core: 128 partitions = (b_loc:2, i:32, jhalf:2), free = (jB:16, k:32).
"""

import sys
import numpy as np

sys.path.insert(0, "/opt/trn_rl_repo")

from contextlib import ExitStack

import concourse.bass as bass
import concourse.tile as tile
from concourse import bacc, mybir
from concourse.bass_utils import run_bass_kernel_spmd

B, N, L = 16, 32, 40
NCORES = 8
B_LOC = B // NCORES  # 2
P = 128  # partitions = B_LOC * N * 2
NJB, NK = 16, 32  # free layout

BOHR = 0.52917721092
ITA = 1.12
ZETA = 4.0
RS_VALS = np.array([0.5, 1.17, 1.83, 2.5, 3.17]) / BOHR
NR, NM = 5, 4

F32 = mybir.dt.float32
I32 = mybir.dt.int32
OP = mybir.AluOpType
ACT = mybir.ActivationFunctionType

# input packing offsets along the free axis of the single [128, 243] input tensor
OFF_CI = 0      # [3]   coords of atom i (per-partition scalars)
OFF_CJ = 3      # [48]  coords of the 16 j-atoms of this partition's half, layout [c][jB]
OFF_CK = 51     # [96]  coords of all 32 k-atoms, layout [c][k]
OFF_UJ = 147    # [16]  d[b,i,j]
OFF_UK = 163    # [32]  d[b,i,k]
OFF_FJ = 195    # [16]  d_cutoff[b,i,j]
OFF_FK = 211    # [32]  d_cutoff[b,i,k]
NIN = 243


def _build():
    nc = bacc.Bacc("TRN2", target_bir_lowering=False, debug=False)
    inp_d = nc.declare_dram_parameter("inp", [P, NIN], F32, isOutput=False)
    cst_d = nc.declare_dram_parameter("cst", [P, 64], F32, isOutput=False)
    out_d = nc.declare_dram_parameter("out", [B_LOC * N, L], F32, isOutput=True)

    TWO_PI = float(2.0 * np.pi)

    with tile.TileContext(nc) as tc, ExitStack() as ctx:
        pool = ctx.enter_context(tc.tile_pool(name="sb", bufs=1))
        rad_pool = ctx.enter_context(tc.tile_pool(name="rad", bufs=2))
        w_pool = ctx.enter_context(tc.tile_pool(name="w", bufs=2))
        scr_pool = ctx.enter_context(tc.tile_pool(name="scr", bufs=2))
        psum = ctx.enter_context(tc.tile_pool(name="ps", bufs=1, space="PSUM"))

        inp = pool.tile([P, NIN], F32, name="inp", tag="inp")
        cst = pool.tile([P, 64], F32, name="cst", tag="cst")
        nc.sync.dma_start(inp[:], inp_d[:])
        nc.sync.dma_start(cst[:], cst_d[:])

        def big(tag, dt=F32):
            return pool.tile([P, NJB, NK], dt, name=tag, tag=tag)

        # ---- small prep: vj = xj - xi (= -v_j), vk = xk - xi (= -v_k); dot = vj.vk ----
        vj = pool.tile([P, 3, NJB], F32, name="vj", tag="vj")
        vk = pool.tile([P, 3, NK], F32, name="vk", tag="vk")
        for c in range(3):
            ci_c = inp[:, OFF_CI + c : OFF_CI + c + 1]
            nc.vector.tensor_scalar(
                vj[:, c, :], inp[:, OFF_CJ + c * NJB : OFF_CJ + (c + 1) * NJB],
                ci_c, None, OP.subtract)
            nc.vector.tensor_scalar(
                vk[:, c, :], inp[:, OFF_CK + c * NK : OFF_CK + (c + 1) * NK],
                ci_c, None, OP.subtract)

        def jview(ap2d):  # [P, 16] -> [P, 16, 32] broadcast along k
            return ap2d.unsqueeze(2).to_broadcast([P, NJB, NK])

        def kview(ap2d):  # [P, 32] -> [P, 16, 32] broadcast along jB
            return ap2d.unsqueeze(1).to_broadcast([P, NJB, NK])

        # ---- dot = sum_c vj_c * vk_c ----
        prod = big("prod")
        dot = big("dot")
        nc.vector.tensor_tensor(dot[:], jview(vj[:, 0, :]), kview(vk[:, 0, :]), OP.mult)
        for c in (1, 2):
            nc.vector.tensor_tensor(prod[:], jview(vj[:, c, :]), kview(vk[:, c, :]), OP.mult)
            nc.vector.tensor_tensor(dot[:], dot[:], prod[:], OP.add)

        # ---- thp = theta / 2pi  (theta = dot / (uj*uk + 1e-5)) ----
        ujv = jview(inp[:, OFF_UJ : OFF_UJ + NJB])
        ukv = kview(inp[:, OFF_UK : OFF_UK + NK])
        den = big("den")
        nc.gpsimd.tensor_tensor(den[:], ujv, ukv, OP.mult)
        nc.vector.tensor_scalar(den[:], den[:], 1e-5, TWO_PI, OP.add, OP.mult)
        rden = big("rden")
        nc.vector.reciprocal_approx_fast(rden[:], den[:])
        thp = big("thp")
        nc.vector.tensor_tensor(thp[:], dot[:], rden[:], OP.mult)

        # ---- range-reduce + sin/cos via Sin table (round-to-nearest int32 convert) ----
        def trig(src, out_tag):
            n_i = pool.tile([P, NJB, NK], I32, name=out_tag + "_n", tag=out_tag + "_n")
            nc.vector.tensor_copy(n_i[:], src[:])
            n_f = big(out_tag + "_nf")
            nc.scalar.activation(n_f[:], n_i[:], ACT.Copy)
            fr = big(out_tag + "_fr")
            nc.gpsimd.tensor_tensor(fr[:], src[:], n_f[:], OP.subtract)
            o = big(out_tag)
            nc.scalar.activation(o[:], fr[:], ACT.Sin, scale=TWO_PI)
            return o

        s1 = trig(thp, "s1")
        r2 = big("r2")
        nc.scalar.activation(r2[:], thp[:], ACT.Copy, bias=0.25)
        c1 = trig(r2, "c1")

        # ---- 4 angular fields (1+-c)^4, (1+-s)^4 via two chained Squares on ScalarE ----
        bias_one = pool.tile([P, 1], F32, name="bias_one", tag="bias_one")
        nc.vector.memset(bias_one[:], 1.0)
        angs = []
        for nm, src, sc in (("ap", c1, 1.0), ("bp", s1, 1.0), ("am", c1, -1.0), ("bm", s1, -1.0)):
            g = big("g_" + nm)
            nc.scalar.activation(g[:], src[:], ACT.Square, bias=bias_one[:], scale=sc)
            a = big("ang_" + nm)
            nc.scalar.activation(a[:], g[:], ACT.Square)
            angs.append(a)

        # ---- radial prep: q = uj + uk; cut = 0.125 * fj * fk  (0.125 = 2^(1-zeta)) ----
        q = big("q")
        nc.gpsimd.tensor_tensor(q[:], ujv, ukv, OP.add)
        cut = big("cut")
        nc.vector.scalar_tensor_tensor(
            cut[:], jview(inp[:, OFF_FJ : OFF_FJ + NJB]), 0.125,
            kview(inp[:, OFF_FK : OFF_FK + NK]), OP.mult, OP.mult)

        rs_bias = pool.tile([P, NR], F32, name="rs_bias", tag="rs_bias")
        for r in range(NR):
            nc.vector.memset(rs_bias[:, r : r + 1], float(-RS_VALS[r]))

        spart = pool.tile([P, 24], F32, name="spart", tag="spart")

        # ---- per radial r: rad = exp(-ita*(q/2 - Rs)^2); W = cut*rad; 4 fused reduces ----
        for r in range(NR):
            sq = rad_pool.tile([P, NJB, NK], F32, name=f"sq{r}", tag="sq")
            nc.scalar.activation(sq[:], q[:], ACT.Square, bias=rs_bias[:, r : r + 1], scale=0.5)
            rad = rad_pool.tile([P, NJB, NK], F32, name=f"rad{r}", tag="rad")
            nc.scalar.activation(rad[:], sq[:], ACT.Exp, scale=float(-ITA))
            W = w_pool.tile([P, NJB, NK], F32, name=f"w{r}", tag="w")
            nc.vector.tensor_tensor(W[:], cut[:], rad[:], OP.mult)
            for m in range(NM):
                scr = scr_pool.tile([P, NJB, NK], F32, name=f"scr{r}{m}", tag="scr")
                nc.vector.scalar_tensor_tensor(
                    scr[:], W[:], 0.0, angs[m][:], OP.bypass, OP.mult,
                    accum_out=spart[:, r * NM + m : r * NM + m + 1])

        # ---- combine: S2[bi, rm] = pair-sum over the two jhalf partitions ----
        s2p = psum.tile([64, NR * NM], F32, name="s2p", tag="s2p")
        nc.tensor.matmul(s2p[:], cst[:, 0:64], spart[:, 0 : NR * NM])

        # ---- output: l = lam*20 + r*4 + t ; lam=+1 -> m=t ; lam=-1 -> m=(t+2)%4 ----
        s2s = pool.tile([64, NR * NM], F32, name="s2s", tag="s2s")
        nc.vector.tensor_copy(s2s[:], s2p[:])
        out3 = out_d[:].rearrange("n (g r t) -> n g r t", g=2, r=NR, t=NM)
        s2v = s2s[:].rearrange("n (r t) -> n r t", r=NR, t=NM)
        nc.sync.dma_start(out3[:, 0, :, :], s2v)                  # lam=+1: direct
        nc.sync.dma_start(out3[:, 1, :, 0:2], s2v[:, :, 2:4])     # lam=-1: swap halves
        nc.sync.dma_start(out3[:, 1, :, 2:4], s2v[:, :, 0:2])

    nc.compile()
    return nc


def _ensure_ntff_hook():
    """Register the axon NTFF profiling hook if the image lacks antenv.axon_hooks."""
    import types

    try:
        from antenv.axon_hooks import get_axon_ntff_profile_hook
        if get_axon_ntff_profile_hook() is not None:
            return
        have_mod = True
    except ImportError:
        have_mod = False
    try:
        if "/root/.axon_site" not in sys.path:
            sys.path.insert(0, "/root/.axon_site")
        from trn_agent_boot.trn_boot import _ntff_profile_via_ctypes

        hook = _ntff_profile_via_ctypes("/opt/axon/libaxon_pjrt.so")
        if hook is None:
            return
    except Exception:
        return
    if have_mod:
        from antenv import axon_hooks
        axon_hooks.set_axon_ntff_profile_hook(hook)
    else:
        m = types.ModuleType("antenv.axon_hooks")
        _h = [hook]
        m.get_axon_ntff_profile_hook = lambda: _h[0]
        m.set_axon_ntff_profile_hook = lambda h: _h.__setitem__(0, h)
        import antenv
        antenv.axon_hooks = m
        sys.modules["antenv.axon_hooks"] = m


_NC = None


def _get_nc():
    global _NC
    if _NC is None:
        _NC = _build()
    return _NC


def _host_pack(d_cutoff, d, atom_coordinates):
    """Pure gather/replication of raw inputs into the per-core [128, 243] layout."""
    d_cutoff = np.ascontiguousarray(d_cutoff, dtype=np.float32)
    d = np.ascontiguousarray(d, dtype=np.float32)
    coords = np.ascontiguousarray(atom_coordinates, dtype=np.float32)

    p = np.arange(P)
    b_of_p = p // (N * 2)
    i_of_p = (p // 2) % N
    jA = p % 2
    jfull = jA[:, None] * NJB + np.arange(NJB)[None, :]  # [P, 16]
    karr = np.arange(NK)

    in_maps = []
    for c in range(NCORES):
        cd = coords[c * B_LOC : (c + 1) * B_LOC]
        dd = d[c * B_LOC : (c + 1) * B_LOC]
        fc = d_cutoff[c * B_LOC : (c + 1) * B_LOC]
        buf = np.empty((P, NIN), dtype=np.float32)
        buf[:, OFF_CI : OFF_CI + 3] = cd[b_of_p, i_of_p]                     # [P,3]
        cjv = cd[b_of_p[:, None], jfull]                                     # [P,16,3]
        buf[:, OFF_CJ : OFF_CJ + 48] = cjv.transpose(0, 2, 1).reshape(P, 48)
        ckv = cd[b_of_p]                                                     # [P,32,3]
        buf[:, OFF_CK : OFF_CK + 96] = ckv.transpose(0, 2, 1).reshape(P, 96)
        buf[:, OFF_UJ : OFF_UJ + 16] = dd[b_of_p[:, None], i_of_p[:, None], jfull]
        buf[:, OFF_UK : OFF_UK + 32] = dd[b_of_p[:, None], i_of_p[:, None], karr[None, :]]
        buf[:, OFF_FJ : OFF_FJ + 16] = fc[b_of_p[:, None], i_of_p[:, None], jfull]
        buf[:, OFF_FK : OFF_FK + 32] = fc[b_of_p[:, None], i_of_p[:, None], karr[None, :]]
        in_maps.append({"inp": buf, "cst": _const_blob()})
    return in_maps


_CST = None


def _const_blob():
    global _CST
    if _CST is None:
        cst = np.zeros((P, 64), dtype=np.float32)
        cst[:, 0:64] = np.repeat(np.eye(64, dtype=np.float32), 2, axis=0)
        _CST = cst
    return _CST


def kernel(d_cutoff, d, atom_coordinates, _trace=False):
    if _trace:
        _ensure_ntff_hook()
    nc = _get_nc()
    in_maps = _host_pack(d_cutoff, d, atom_coordinates)
    res = run_bass_kernel_spmd(nc, in_maps, core_ids=list(range(NCORES)), trace=_trace)
    out = np.concatenate(
        [res.results[c]["out"].reshape(B_LOC, N, L) for c in range(NCORES)], axis=0
    ).astype(np.float32)
    if _trace:
        kernel._last_results = res
    return out


# revision 8
# speedup vs baseline: 1.5719x; 1.2187x over previous
"""Trainium2 Bass kernel for AngularSymmetryMod (ANI-style angular symmetry functions).

Math: out[b,i,l] = sum_{j,k} (1+lam*cos(theta-theta_t))^zeta * exp(-ita*((R_ij+R_ik)/2-Rs)^2)
                            * f_ij*f_ik * 2^(1-zeta)
over a 40-point parameter grid l=(lam in {+-1}, 5 Rs values, 4 theta_t values), zeta=4.

Key optimizations:
 1. theta_t = {0.0, 1.57, 3.14, 4.71} are (to 8e-4) the exact quadrants {0, pi/2, pi, 3pi/2},
    so cos(theta-theta_t) = {c, s, -c, -s} and the angular factor collapses to FOUR distinct
    fields: (1+-c)^4, (1+-s)^4 — each two chained Square activations on the ScalarEngine.
    (Validated: 2.2e-4 rel err vs the f32 reference.)
 2. Each of the 40 outputs is S[r, m] (5 radials x 4 angulars = 20 reductions); the 40 outputs
    are a column remap handled by the output DMA.
 3. The (j,k) summand is symmetric, so only the 528 pairs j<=k are computed (host gathers the
    packed pair layout; off-diagonal weight 2 is folded into the cutoff product on-chip).
 4. sin/cos need exact-range reduction (theta spans +-2.3e6): theta/2pi - round(theta/2pi) via
    the DVE's round-to-nearest f32->int32 convert, then the Sin table on [-pi, pi].

Sharding: data-parallel over batch (16 molecules -> 2 per core on 8 cores). No collectives.
Layout per core: 128 partitions = (b_loc:2, i:32, half:2), free = 264 packed (j,k) pairs
(248 off-diagonal + 16 diagonal per half).
"""

import sys
import numpy as np

sys.path.insert(0, "/opt/trn_rl_repo")

from contextlib import ExitStack

import concourse.bass as bass
import concourse.tile as tile
from concourse import bacc, mybir
from concourse.bass_utils import run_bass_kernel_spmd

B, N, L = 16, 32, 40
NCORES = 8
B_LOC = B // NCORES  # 2
P = 128  # partitions = B_LOC * N * 2
NT = 264           # packed pairs per partition-half
NOFF = 248         # off-diagonal entries (first NOFF of NT); rest are diagonal

BOHR = 0.52917721092
ITA = 1.12
ZETA = 4.0
RS_VALS = np.array([0.5, 1.17, 1.83, 2.5, 3.17]) / BOHR
NR, NM = 5, 4

F32 = mybir.dt.float32
I32 = mybir.dt.int32
OP = mybir.AluOpType
ACT = mybir.ActivationFunctionType

# free-axis offsets in the single packed [128, NIN] input tensor
OFF_CI = 0                 # [3]       coords of atom i (per-partition scalars)
OFF_CJ = 3                 # [3*NT]    coords of j-atom of pair t, layout [c][t]
OFF_CK = 3 + 3 * NT        # [3*NT]    coords of k-atom of pair t
OFF_UJ = 3 + 6 * NT        # [NT]      d[b,i,j_t]
OFF_UK = OFF_UJ + NT       # [NT]      d[b,i,k_t]
OFF_FJ = OFF_UK + NT       # [NT]      d_cutoff[b,i,j_t]
OFF_FK = OFF_FJ + NT       # [NT]      d_cutoff[b,i,k_t]
NIN = 3 + 10 * NT


def _pair_index():
    """Static (j,k) pair enumeration: per half, 248 off-diagonal + 16 diagonal."""
    pairs = [(j, k) for j in range(N) for k in range(j + 1, N)]  # 496
    halves = [pairs[0::2], pairs[1::2]]
    tri_j = np.zeros((2, NT), dtype=np.int64)
    tri_k = np.zeros((2, NT), dtype=np.int64)
    for h in range(2):
        for t, (j, k) in enumerate(halves[h]):
            tri_j[h, t], tri_k[h, t] = j, k
        for t2, j in enumerate(range(h * 16, (h + 1) * 16)):
            tri_j[h, NOFF + t2] = tri_k[h, NOFF + t2] = j
    return tri_j, tri_k


_TRI_J, _TRI_K = _pair_index()


def _build():
    nc = bacc.Bacc("TRN2", target_bir_lowering=False, debug=False)
    inp_d = nc.declare_dram_parameter("inp", [P, NIN], F32, isOutput=False)
    cst_d = nc.declare_dram_parameter("cst", [P, 64], F32, isOutput=False)
    out_d = nc.declare_dram_parameter("out", [B_LOC * N, L], F32, isOutput=True)

    TWO_PI = float(2.0 * np.pi)

    with tile.TileContext(nc) as tc, ExitStack() as ctx:
        pool = ctx.enter_context(tc.tile_pool(name="sb", bufs=1))
        rad_pool = ctx.enter_context(tc.tile_pool(name="rad", bufs=2))
        w_pool = ctx.enter_context(tc.tile_pool(name="w", bufs=2))
        scr_pool = ctx.enter_context(tc.tile_pool(name="scr", bufs=3))
        psum = ctx.enter_context(tc.tile_pool(name="ps", bufs=1, space="PSUM"))

        def big(tag, dt=F32):
            return pool.tile([P, NT], dt, name=tag, tag=tag)

        # split input DMAs so consumers start as soon as their slice lands
        cj = pool.tile([P, 3, NT], F32, name="cj", tag="cj")
        ck = pool.tile([P, 3, NT], F32, name="ck", tag="ck")
        ci = pool.tile([P, 3], F32, name="ci", tag="ci")
        uf = pool.tile([P, 4, NT], F32, name="uf", tag="uf")
        cst = pool.tile([P, 64], F32, name="cst", tag="cst")
        nc.sync.dma_start(ci[:], inp_d[:, OFF_CI : OFF_CI + 3])
        nc.sync.dma_start(cj[:], inp_d[:, OFF_CJ : OFF_CJ + 3 * NT].rearrange("p (c t) -> p c t", c=3))
        nc.sync.dma_start(ck[:], inp_d[:, OFF_CK : OFF_CK + 3 * NT].rearrange("p (c t) -> p c t", c=3))
        nc.sync.dma_start(uf[:], inp_d[:, OFF_UJ : OFF_UJ + 4 * NT].rearrange("p (c t) -> p c t", c=4))
        nc.sync.dma_start(cst[:], cst_d[:])
        uj, uk, fj, fk = uf[:, 0, :], uf[:, 1, :], uf[:, 2, :], uf[:, 3, :]

        # ---- vj = xj - xi (= -v_j), vk = xk - xi (= -v_k); dot = vj.vk ----
        vj = pool.tile([P, 3, NT], F32, name="vj", tag="vj")
        vk = pool.tile([P, 3, NT], F32, name="vk", tag="vk")
        for c in range(3):
            nc.vector.tensor_scalar(vj[:, c, :], cj[:, c, :], ci[:, c : c + 1], None, OP.subtract)
            nc.vector.tensor_scalar(vk[:, c, :], ck[:, c, :], ci[:, c : c + 1], None, OP.subtract)

        prod = big("prod")
        dot = big("dot")
        nc.vector.tensor_tensor(dot[:], vj[:, 0, :], vk[:, 0, :], OP.mult)
        for c in (1, 2):
            nc.vector.tensor_tensor(prod[:], vj[:, c, :], vk[:, c, :], OP.mult)
            nc.vector.tensor_tensor(dot[:], dot[:], prod[:], OP.add)

        # ---- thp = theta / 2pi  (theta = dot / (uj*uk + 1e-5)) ----
        den = big("den")
        nc.gpsimd.tensor_tensor(den[:], uj, uk, OP.mult)
        nc.vector.tensor_scalar(den[:], den[:], 1e-5, TWO_PI, OP.add, OP.mult)
        rden = big("rden")
        nc.vector.reciprocal_approx_fast(rden[:], den[:])
        thp = big("thp")
        nc.vector.tensor_tensor(thp[:], dot[:], rden[:], OP.mult)

        # ---- range-reduce + sin/cos via Sin table (round-to-nearest int32 convert) ----
        def trig(src, out_tag):
            n_i = pool.tile([P, NT], I32, name=out_tag + "_n", tag=out_tag + "_n")
            nc.vector.tensor_copy(n_i[:], src[:])
            n_f = big(out_tag + "_nf")
            nc.scalar.activation(n_f[:], n_i[:], ACT.Copy)
            fr = big(out_tag + "_fr")
            nc.gpsimd.tensor_tensor(fr[:], src[:], n_f[:], OP.subtract)
            o = big(out_tag)
            nc.scalar.activation(o[:], fr[:], ACT.Sin, scale=TWO_PI)
            return o

        s1 = trig(thp, "s1")
        r2 = big("r2")
        nc.scalar.activation(r2[:], thp[:], ACT.Copy, bias=0.25)
        c1 = trig(r2, "c1")

        # ---- 4 angular fields (1+-c)^4, (1+-s)^4 via two chained Squares on ScalarE ----
        bias_one = pool.tile([P, 1], F32, name="bias_one", tag="bias_one")
        nc.vector.memset(bias_one[:], 1.0)
        angs = []
        for nm, src, sc in (("ap", c1, 1.0), ("bp", s1, 1.0), ("am", c1, -1.0), ("bm", s1, -1.0)):
            g = big("g_" + nm)
            nc.scalar.activation(g[:], src[:], ACT.Square, bias=bias_one[:], scale=sc)
            a = big("ang_" + nm)
            nc.scalar.activation(a[:], g[:], ACT.Square)
            angs.append(a)

        # ---- radial prep: q = uj + uk; cut = w * fj * fk, w = 0.25 offdiag / 0.125 diag ----
        q = big("q")
        nc.gpsimd.tensor_tensor(q[:], uj, uk, OP.add)
        cut = big("cut")
        nc.vector.scalar_tensor_tensor(
            cut[:, :NOFF], fj[:, :NOFF], 0.25, fk[:, :NOFF], OP.mult, OP.mult)
        nc.vector.scalar_tensor_tensor(
            cut[:, NOFF:], fj[:, NOFF:], 0.125, fk[:, NOFF:], OP.mult, OP.mult)

        rs_bias = pool.tile([P, NR], F32, name="rs_bias", tag="rs_bias")
        for r in range(NR):
            nc.vector.memset(rs_bias[:, r : r + 1], float(-RS_VALS[r]))

        spart = pool.tile([P, 24], F32, name="spart", tag="spart")

        # ---- per radial r: rad = exp(-ita*(q/2 - Rs)^2); W = cut*rad; 4 fused reduces ----
        GPS_REDUCE = set()  # (r, m) pairs on gpsimd
        for r in range(NR):
            sq = rad_pool.tile([P, NT], F32, name=f"sq{r}", tag="sq")
            nc.scalar.activation(sq[:], q[:], ACT.Square, bias=rs_bias[:, r : r + 1], scale=0.5)
            rad = rad_pool.tile([P, NT], F32, name=f"rad{r}", tag="rad")
            nc.scalar.activation(rad[:], sq[:], ACT.Exp, scale=float(-ITA))
            W = w_pool.tile([P, NT], F32, name=f"w{r}", tag="w")
            nc.vector.tensor_tensor(W[:], cut[:], rad[:], OP.mult)
            for m in range(NM):
                eng = nc.gpsimd if (r, m) in GPS_REDUCE else nc.vector
                scr = scr_pool.tile([P, NT], F32, name=f"scr{r}{m}",
                                    tag="gscr" if (r, m) in GPS_REDUCE else "scr")
                eng.scalar_tensor_tensor(
                    scr[:], W[:], 0.0, angs[m][:], OP.bypass, OP.mult,
                    accum_out=spart[:, r * NM + m : r * NM + m + 1])

        # ---- combine: S2[bi, rm] = pair-sum over the two half partitions ----
        s2p = psum.tile([64, NR * NM], F32, name="s2p", tag="s2p")
        nc.tensor.matmul(s2p[:], cst[:, 0:64], spart[:, 0 : NR * NM])
        s2s = pool.tile([64, NR * NM], F32, name="s2s", tag="s2s")
        nc.vector.tensor_copy(s2s[:], s2p[:])

        # ---- output: l = lam*20 + r*4 + t ; lam=+1 -> m=t ; lam=-1 -> m=(t+2)%4 ----
        out3 = out_d[:].rearrange("n (g r t) -> n g r t", g=2, r=NR, t=NM)
        s2v = s2s[:].rearrange("n (r t) -> n r t", r=NR, t=NM)
        nc.sync.dma_start(out3[:, 0, :, :], s2v)                  # lam=+1: direct
        nc.sync.dma_start(out3[:, 1, :, 0:2], s2v[:, :, 2:4])     # lam=-1: swap halves
        nc.sync.dma_start(out3[:, 1, :, 2:4], s2v[:, :, 0:2])

    nc.compile()
    return nc


def _ensure_ntff_hook():
    """Register the axon NTFF profiling hook if the image lacks antenv.axon_hooks."""
    import types

    try:
        from antenv.axon_hooks import get_axon_ntff_profile_hook
        if get_axon_ntff_profile_hook() is not None:
            return
        have_mod = True
    except ImportError:
        have_mod = False
    try:
        if "/root/.axon_site" not in sys.path:
            sys.path.insert(0, "/root/.axon_site")
        from trn_agent_boot.trn_boot import _ntff_profile_via_ctypes

        hook = _ntff_profile_via_ctypes("/opt/axon/libaxon_pjrt.so")
        if hook is None:
            return
    except Exception:
        return
    if have_mod:
        from antenv import axon_hooks
        axon_hooks.set_axon_ntff_profile_hook(hook)
    else:
        m = types.ModuleType("antenv.axon_hooks")
        _h = [hook]
        m.get_axon_ntff_profile_hook = lambda: _h[0]
        m.set_axon_ntff_profile_hook = lambda h: _h.__setitem__(0, h)
        import antenv
        antenv.axon_hooks = m
        sys.modules["antenv.axon_hooks"] = m


_NC = None


def _get_nc():
    global _NC
    if _NC is None:
        _NC = _build()
    return _NC


def _host_pack(d_cutoff, d, atom_coordinates):
    """Pure gather/replication of raw inputs into the per-core packed layout."""
    d_cutoff = np.ascontiguousarray(d_cutoff, dtype=np.float32)
    d = np.ascontiguousarray(d, dtype=np.float32)
    coords = np.ascontiguousarray(atom_coordinates, dtype=np.float32)

    p = np.arange(P)
    b_of_p = p // (N * 2)          # [P]
    i_of_p = (p // 2) % N          # [P]
    half = p % 2                   # [P]
    jt = _TRI_J[half]              # [P, NT]
    kt = _TRI_K[half]              # [P, NT]

    in_maps = []
    for c in range(NCORES):
        cd = coords[c * B_LOC : (c + 1) * B_LOC]
        dd = d[c * B_LOC : (c + 1) * B_LOC]
        fc = d_cutoff[c * B_LOC : (c + 1) * B_LOC]
        buf = np.empty((P, NIN), dtype=np.float32)
        buf[:, OFF_CI : OFF_CI + 3] = cd[b_of_p, i_of_p]
        buf[:, OFF_CJ : OFF_CJ + 3 * NT] = (
            cd[b_of_p[:, None], jt].transpose(0, 2, 1).reshape(P, 3 * NT))
        buf[:, OFF_CK : OFF_CK + 3 * NT] = (
            cd[b_of_p[:, None], kt].transpose(0, 2, 1).reshape(P, 3 * NT))
        buf[:, OFF_UJ : OFF_UJ + NT] = dd[b_of_p[:, None], i_of_p[:, None], jt]
        buf[:, OFF_UK : OFF_UK + NT] = dd[b_of_p[:, None], i_of_p[:, None], kt]
        buf[:, OFF_FJ : OFF_FJ + NT] = fc[b_of_p[:, None], i_of_p[:, None], jt]
        buf[:, OFF_FK : OFF_FK + NT] = fc[b_of_p[:, None], i_of_p[:, None], kt]
        in_maps.append({"inp": buf, "cst": _const_blob()})
    return in_maps


_CST = None


def _const_blob():
    global _CST
    if _CST is None:
        cst = np.zeros((P, 64), dtype=np.float32)
        cst[:, 0:64] = np.repeat(np.eye(64, dtype=np.float32), 2, axis=0)
        _CST = cst
    return _CST


def kernel(d_cutoff, d, atom_coordinates, _trace=False):
    if _trace:
        _ensure_ntff_hook()
    nc = _get_nc()
    in_maps = _host_pack(d_cutoff, d, atom_coordinates)
    res = run_bass_kernel_spmd(nc, in_maps, core_ids=list(range(NCORES)), trace=_trace)
    out = np.concatenate(
        [res.results[c]["out"].reshape(B_LOC, N, L) for c in range(NCORES)], axis=0
    ).astype(np.float32)
    if _trace:
        kernel._last_results = res
    return out


# revision 9
# speedup vs baseline: 1.5792x; 1.0047x over previous
"""Trainium2 Bass kernel for AngularSymmetryMod (ANI-style angular symmetry functions).

Math: out[b,i,l] = sum_{j,k} (1+lam*cos(theta-theta_t))^zeta * exp(-ita*((R_ij+R_ik)/2-Rs)^2)
                            * f_ij*f_ik * 2^(1-zeta)
over a 40-point parameter grid l=(lam in {+-1}, 5 Rs values, 4 theta_t values), zeta=4.

Key optimizations:
 1. theta_t = {0.0, 1.57, 3.14, 4.71} are (to 8e-4) the exact quadrants {0, pi/2, pi, 3pi/2},
    so cos(theta-theta_t) = {c, s, -c, -s} and the angular factor collapses to FOUR distinct
    fields: (1+-c)^4, (1+-s)^4 — each two chained Square activations on the ScalarEngine.
    (Validated: 2.2e-4 rel err vs the f32 reference.)
 2. Each of the 40 outputs is S[r, m] (5 radials x 4 angulars = 20 reductions); the 40 outputs
    are a column remap handled by the output DMA.
 3. The (j,k) summand is symmetric, so only the 528 pairs j<=k are computed (host gathers the
    packed pair layout; off-diagonal weight 2 is folded into the cutoff product on-chip).
 4. sin/cos need exact-range reduction (theta spans +-2.3e6): theta/2pi - round(theta/2pi) via
    the DVE's round-to-nearest f32->int32 convert, then the Sin table on [-pi, pi].

Sharding: data-parallel over batch (16 molecules -> 2 per core on 8 cores). No collectives.
Layout per core: 128 partitions = (b_loc:2, i:32, half:2), free = 264 packed (j,k) pairs
(248 off-diagonal + 16 diagonal per half).
"""

import sys
import numpy as np

sys.path.insert(0, "/opt/trn_rl_repo")

from contextlib import ExitStack

import concourse.bass as bass
import concourse.tile as tile
from concourse import bacc, mybir
from concourse.bass_utils import run_bass_kernel_spmd

B, N, L = 16, 32, 40
NCORES = 8
B_LOC = B // NCORES  # 2
P = 128  # partitions = B_LOC * N * 2
NT = 264           # packed pairs per partition-half
NOFF = 248         # off-diagonal entries (first NOFF of NT); rest are diagonal

BOHR = 0.52917721092
ITA = 1.12
ZETA = 4.0
RS_VALS = np.array([0.5, 1.17, 1.83, 2.5, 3.17]) / BOHR
NR, NM = 5, 4

F32 = mybir.dt.float32
I32 = mybir.dt.int32
OP = mybir.AluOpType
ACT = mybir.ActivationFunctionType

# free-axis offsets in the single packed [128, NIN] input tensor
OFF_CI = 0                 # [3]       coords of atom i (per-partition scalars)
OFF_CJ = 3                 # [3*NT]    coords of j-atom of pair t, layout [c][t]
OFF_CK = 3 + 3 * NT        # [3*NT]    coords of k-atom of pair t
OFF_UJ = 3 + 6 * NT        # [NT]      d[b,i,j_t]
OFF_UK = OFF_UJ + NT       # [NT]      d[b,i,k_t]
OFF_FJ = OFF_UK + NT       # [NT]      d_cutoff[b,i,j_t]
OFF_FK = OFF_FJ + NT       # [NT]      d_cutoff[b,i,k_t]
NIN = 3 + 10 * NT


def _pair_index():
    """Static (j,k) pair enumeration: per half, 248 off-diagonal + 16 diagonal."""
    pairs = [(j, k) for j in range(N) for k in range(j + 1, N)]  # 496
    halves = [pairs[0::2], pairs[1::2]]
    tri_j = np.zeros((2, NT), dtype=np.int64)
    tri_k = np.zeros((2, NT), dtype=np.int64)
    for h in range(2):
        for t, (j, k) in enumerate(halves[h]):
            tri_j[h, t], tri_k[h, t] = j, k
        for t2, j in enumerate(range(h * 16, (h + 1) * 16)):
            tri_j[h, NOFF + t2] = tri_k[h, NOFF + t2] = j
    return tri_j, tri_k


_TRI_J, _TRI_K = _pair_index()


def _build():
    nc = bacc.Bacc("TRN2", target_bir_lowering=False, debug=False)
    inp_d = nc.declare_dram_parameter("inp", [P, NIN], F32, isOutput=False)
    cst_d = nc.declare_dram_parameter("cst", [P, 64], F32, isOutput=False)
    out_d = nc.declare_dram_parameter("out", [B_LOC * N, L], F32, isOutput=True)

    TWO_PI = float(2.0 * np.pi)

    with tile.TileContext(nc) as tc, ExitStack() as ctx:
        pool = ctx.enter_context(tc.tile_pool(name="sb", bufs=1))
        rad_pool = ctx.enter_context(tc.tile_pool(name="rad", bufs=2))
        w_pool = ctx.enter_context(tc.tile_pool(name="w", bufs=2))
        scr_pool = ctx.enter_context(tc.tile_pool(name="scr", bufs=3))
        psum = ctx.enter_context(tc.tile_pool(name="ps", bufs=1, space="PSUM"))

        def big(tag, dt=F32):
            return pool.tile([P, NT], dt, name=tag, tag=tag)

        # split input DMAs so consumers start as soon as their slice lands
        cj = pool.tile([P, 3, NT], F32, name="cj", tag="cj")
        ck = pool.tile([P, 3, NT], F32, name="ck", tag="ck")
        ci = pool.tile([P, 3], F32, name="ci", tag="ci")
        uf = pool.tile([P, 4, NT], F32, name="uf", tag="uf")
        cst = pool.tile([P, 64], F32, name="cst", tag="cst")
        nc.sync.dma_start(ci[:], inp_d[:, OFF_CI : OFF_CI + 3])
        nc.sync.dma_start(cj[:], inp_d[:, OFF_CJ : OFF_CJ + 3 * NT].rearrange("p (c t) -> p c t", c=3))
        nc.sync.dma_start(ck[:], inp_d[:, OFF_CK : OFF_CK + 3 * NT].rearrange("p (c t) -> p c t", c=3))
        nc.sync.dma_start(uf[:], inp_d[:, OFF_UJ : OFF_UJ + 4 * NT].rearrange("p (c t) -> p c t", c=4))
        nc.sync.dma_start(cst[:], cst_d[:])
        uj, uk, fj, fk = uf[:, 0, :], uf[:, 1, :], uf[:, 2, :], uf[:, 3, :]

        # ---- vj = xj - xi (= -v_j), vk = xk - xi (= -v_k); dot = vj.vk ----
        vj = pool.tile([P, 3, NT], F32, name="vj", tag="vj")
        vk = pool.tile([P, 3, NT], F32, name="vk", tag="vk")
        for c in range(3):
            nc.vector.tensor_scalar(vj[:, c, :], cj[:, c, :], ci[:, c : c + 1], None, OP.subtract)
            nc.vector.tensor_scalar(vk[:, c, :], ck[:, c, :], ci[:, c : c + 1], None, OP.subtract)

        prod = big("prod")
        dot = big("dot")
        nc.vector.tensor_tensor(dot[:], vj[:, 0, :], vk[:, 0, :], OP.mult)
        for c in (1, 2):
            nc.vector.tensor_tensor(prod[:], vj[:, c, :], vk[:, c, :], OP.mult)
            nc.vector.tensor_tensor(dot[:], dot[:], prod[:], OP.add)

        # ---- thp = theta / 2pi  (theta = dot / (uj*uk + 1e-5)) ----
        den = big("den")
        nc.gpsimd.tensor_tensor(den[:], uj, uk, OP.mult)
        nc.vector.tensor_scalar(den[:], den[:], 1e-5, TWO_PI, OP.add, OP.mult)
        rden = big("rden")
        nc.vector.reciprocal_approx_fast(rden[:], den[:])
        thp = big("thp")
        nc.vector.tensor_tensor(thp[:], dot[:], rden[:], OP.mult)

        # ---- radial stage (emitted before trig so ACT groups Exp with Square:
        #      exp_and_friends loads once, then trig_and_small once) ----
        q = big("q")
        nc.gpsimd.tensor_tensor(q[:], uj, uk, OP.add)
        cut = big("cut")
        nc.vector.scalar_tensor_tensor(
            cut[:, :NOFF], fj[:, :NOFF], 0.25, fk[:, :NOFF], OP.mult, OP.mult)
        nc.vector.scalar_tensor_tensor(
            cut[:, NOFF:], fj[:, NOFF:], 0.125, fk[:, NOFF:], OP.mult, OP.mult)
        rs_bias = pool.tile([P, NR], F32, name="rs_bias", tag="rs_bias")
        for r in range(NR):
            nc.vector.memset(rs_bias[:, r : r + 1], float(-RS_VALS[r]))
        Ws = []
        for r in range(NR):
            sq = rad_pool.tile([P, NT], F32, name=f"sq{r}", tag="sq")
            nc.scalar.activation(sq[:], q[:], ACT.Square, bias=rs_bias[:, r : r + 1], scale=0.5)
            rad = rad_pool.tile([P, NT], F32, name=f"rad{r}", tag="rad")
            nc.scalar.activation(rad[:], sq[:], ACT.Exp, scale=float(-ITA))
            W = w_pool.tile([P, NT], F32, name=f"w{r}", tag=f"w{r}")
            nc.gpsimd.tensor_tensor(W[:], cut[:], rad[:], OP.mult)
            Ws.append(W)

        # ---- range-reduce + sin/cos via Sin table ----
        # round(r) for |r| < 2^22 via the f32 magic constant: (r + 1.5*2^23) - 1.5*2^23
        RC = float(12582912.0)

        def trig(src, out_tag):
            n_f = big(out_tag + "_nf")
            nc.vector.tensor_scalar(n_f[:], src[:], RC, RC, OP.add, OP.subtract)
            fr = big(out_tag + "_fr")
            nc.gpsimd.tensor_tensor(fr[:], src[:], n_f[:], OP.subtract)
            o = big(out_tag)
            nc.scalar.activation(o[:], fr[:], ACT.Sin, scale=TWO_PI)
            return o

        s1 = trig(thp, "s1")
        r2 = big("r2")
        nc.scalar.activation(r2[:], thp[:], ACT.Copy, bias=0.25)
        c1 = trig(r2, "c1")

        # ---- 4 angular fields (1+-c)^4, (1+-s)^4 via two chained Squares on ScalarE ----
        bias_one = pool.tile([P, 1], F32, name="bias_one", tag="bias_one")
        nc.vector.memset(bias_one[:], 1.0)
        angs = []
        for nm, src, sc in (("ap", c1, 1.0), ("bp", s1, 1.0), ("am", c1, -1.0), ("bm", s1, -1.0)):
            g = big("g_" + nm)
            nc.scalar.activation(g[:], src[:], ACT.Square, bias=bias_one[:], scale=sc)
            a = big("ang_" + nm)
            nc.scalar.activation(a[:], g[:], ACT.Square)
            angs.append(a)


        spart = pool.tile([P, 24], F32, name="spart", tag="spart")

        # ---- per radial r: 4 fused reduces sum_t W_r * ang_m ----
        for r in range(NR):
            for m in range(NM):
                scr = scr_pool.tile([P, NT], F32, name=f"scr{r}{m}", tag="scr")
                nc.vector.scalar_tensor_tensor(
                    scr[:], Ws[r][:], 0.0, angs[m][:], OP.bypass, OP.mult,
                    accum_out=spart[:, r * NM + m : r * NM + m + 1])

        # ---- combine: pair-sum over half partitions; assemble all 40 l-columns in PSUM
        #      l = lam*20 + r*4 + t ; lam=+1 -> m=t ; lam=-1 -> m=(t+2)%4 ----
        s2p = psum.tile([64, L], F32, name="s2p", tag="s2p")
        sp3 = spart[:, 0 : NR * NM].rearrange("p (r t) -> p r t", r=NR, t=NM)
        nc.tensor.matmul(s2p[:, 0:20], cst[:, 0:64], spart[:, 0 : NR * NM])
        o3 = s2p[:].rearrange("n (g r t) -> n g r t", g=2, r=NR, t=NM)
        nc.tensor.matmul(o3[:, 1, :, 0:2], cst[:, 0:64], sp3[:, :, 2:4])
        nc.tensor.matmul(o3[:, 1, :, 2:4], cst[:, 0:64], sp3[:, :, 0:2])
        s2s = pool.tile([64, L], F32, name="s2s", tag="s2s")
        nc.vector.tensor_copy(s2s[:], s2p[:])
        nc.sync.dma_start(out_d[:], s2s[:])

    nc.compile()
    return nc


def _ensure_ntff_hook():
    """Register the axon NTFF profiling hook if the image lacks antenv.axon_hooks."""
    import types

    try:
        from antenv.axon_hooks import get_axon_ntff_profile_hook
        if get_axon_ntff_profile_hook() is not None:
            return
        have_mod = True
    except ImportError:
        have_mod = False
    try:
        if "/root/.axon_site" not in sys.path:
            sys.path.insert(0, "/root/.axon_site")
        from trn_agent_boot.trn_boot import _ntff_profile_via_ctypes

        hook = _ntff_profile_via_ctypes("/opt/axon/libaxon_pjrt.so")
        if hook is None:
            return
    except Exception:
        return
    if have_mod:
        from antenv import axon_hooks
        axon_hooks.set_axon_ntff_profile_hook(hook)
    else:
        m = types.ModuleType("antenv.axon_hooks")
        _h = [hook]
        m.get_axon_ntff_profile_hook = lambda: _h[0]
        m.set_axon_ntff_profile_hook = lambda h: _h.__setitem__(0, h)
        import antenv
        antenv.axon_hooks = m
        sys.modules["antenv.axon_hooks"] = m


_NC = None


def _get_nc():
    global _NC
    if _NC is None:
        _NC = _build()
    return _NC


def _host_pack(d_cutoff, d, atom_coordinates):
    """Pure gather/replication of raw inputs into the per-core packed layout."""
    d_cutoff = np.ascontiguousarray(d_cutoff, dtype=np.float32)
    d = np.ascontiguousarray(d, dtype=np.float32)
    coords = np.ascontiguousarray(atom_coordinates, dtype=np.float32)

    p = np.arange(P)
    b_of_p = p // (N * 2)          # [P]
    i_of_p = (p // 2) % N          # [P]
    half = p % 2                   # [P]
    jt = _TRI_J[half]              # [P, NT]
    kt = _TRI_K[half]              # [P, NT]

    in_maps = []
    for c in range(NCORES):
        cd = coords[c * B_LOC : (c + 1) * B_LOC]
        dd = d[c * B_LOC : (c + 1) * B_LOC]
        fc = d_cutoff[c * B_LOC : (c + 1) * B_LOC]
        buf = np.empty((P, NIN), dtype=np.float32)
        buf[:, OFF_CI : OFF_CI + 3] = cd[b_of_p, i_of_p]
        buf[:, OFF_CJ : OFF_CJ + 3 * NT] = (
            cd[b_of_p[:, None], jt].transpose(0, 2, 1).reshape(P, 3 * NT))
        buf[:, OFF_CK : OFF_CK + 3 * NT] = (
            cd[b_of_p[:, None], kt].transpose(0, 2, 1).reshape(P, 3 * NT))
        buf[:, OFF_UJ : OFF_UJ + NT] = dd[b_of_p[:, None], i_of_p[:, None], jt]
        buf[:, OFF_UK : OFF_UK + NT] = dd[b_of_p[:, None], i_of_p[:, None], kt]
        buf[:, OFF_FJ : OFF_FJ + NT] = fc[b_of_p[:, None], i_of_p[:, None], jt]
        buf[:, OFF_FK : OFF_FK + NT] = fc[b_of_p[:, None], i_of_p[:, None], kt]
        in_maps.append({"inp": buf, "cst": _const_blob()})
    return in_maps


_CST = None


def _const_blob():
    global _CST
    if _CST is None:
        cst = np.zeros((P, 64), dtype=np.float32)
        cst[:, 0:64] = np.repeat(np.eye(64, dtype=np.float32), 2, axis=0)
        _CST = cst
    return _CST


def kernel(d_cutoff, d, atom_coordinates, _trace=False):
    if _trace:
        _ensure_ntff_hook()
    nc = _get_nc()
    in_maps = _host_pack(d_cutoff, d, atom_coordinates)
    res = run_bass_kernel_spmd(nc, in_maps, core_ids=list(range(NCORES)), trace=_trace)
    out = np.concatenate(
        [res.results[c]["out"].reshape(B_LOC, N, L) for c in range(NCORES)], axis=0
    ).astype(np.float32)
    if _trace:
        kernel._last_results = res
    return out


# revision 12
# speedup vs baseline: 1.6232x; 1.0278x over previous
"""Trainium2 Bass kernel for AngularSymmetryMod (ANI-style angular symmetry functions).

Math: out[b,i,l] = sum_{j,k} (1+lam*cos(theta-theta_t))^zeta * exp(-ita*((R_ij+R_ik)/2-Rs)^2)
                            * f_ij*f_ik * 2^(1-zeta)
over a 40-point parameter grid l=(lam in {+-1}, 5 Rs values, 4 theta_t values), zeta=4.

Key optimizations:
 1. theta_t = {0.0, 1.57, 3.14, 4.71} are (to 8e-4) the exact quadrants {0, pi/2, pi, 3pi/2},
    so cos(theta-theta_t) = {c, s, -c, -s} and the angular factor collapses to FOUR distinct
    fields: (1+-c)^4, (1+-s)^4 — each two chained Square activations on the ScalarEngine.
    (Validated: 2.2e-4 rel err vs the f32 reference.)
 2. Each of the 40 outputs is S[r, m] (5 radials x 4 angulars = 20 reductions); the 40 outputs
    are a column remap handled by the output DMA.
 3. The (j,k) summand is symmetric, so only the 528 pairs j<=k are computed (host gathers the
    packed pair layout; off-diagonal weight 2 is folded into the cutoff product on-chip).
 4. sin/cos need exact-range reduction (theta spans +-2.3e6): theta/2pi - round(theta/2pi) via
    the DVE's round-to-nearest f32->int32 convert, then the Sin table on [-pi, pi].

Sharding: data-parallel over batch (16 molecules -> 2 per core on 8 cores). No collectives.
Layout per core: 128 partitions = (b_loc:2, i:32, half:2), free = 264 packed (j,k) pairs
(248 off-diagonal + 16 diagonal per half).
"""

import sys
import numpy as np

sys.path.insert(0, "/opt/trn_rl_repo")

from contextlib import ExitStack

import concourse.bass as bass
import concourse.tile as tile
from concourse import bacc, mybir
from concourse.bass_utils import run_bass_kernel_spmd

B, N, L = 16, 32, 40
NCORES = 8
B_LOC = B // NCORES  # 2
P = 128  # partitions = B_LOC * N * 2
NT = 264           # packed pairs per partition-half
NOFF = 248         # off-diagonal entries (first NOFF of NT); rest are diagonal

BOHR = 0.52917721092
ITA = 1.12
ZETA = 4.0
RS_VALS = np.array([0.5, 1.17, 1.83, 2.5, 3.17]) / BOHR
NR, NM = 5, 4

F32 = mybir.dt.float32
I32 = mybir.dt.int32
OP = mybir.AluOpType
ACT = mybir.ActivationFunctionType

# free-axis offsets in the single packed [128, NIN] input tensor
OFF_CI = 0                 # [3]       coords of atom i (per-partition scalars)
OFF_CJ = 3                 # [3*NT]    coords of j-atom of pair t, layout [c][t]
OFF_CK = 3 + 3 * NT        # [3*NT]    coords of k-atom of pair t
OFF_UJ = 3 + 6 * NT        # [NT]      d[b,i,j_t]
OFF_UK = OFF_UJ + NT       # [NT]      d[b,i,k_t]
OFF_FJ = OFF_UK + NT       # [NT]      d_cutoff[b,i,j_t]
OFF_FK = OFF_FJ + NT       # [NT]      d_cutoff[b,i,k_t]
NIN = 3 + 10 * NT


def _pair_index():
    """Static (j,k) pair enumeration: per half, 248 off-diagonal + 16 diagonal."""
    pairs = [(j, k) for j in range(N) for k in range(j + 1, N)]  # 496
    halves = [pairs[0::2], pairs[1::2]]
    tri_j = np.zeros((2, NT), dtype=np.int64)
    tri_k = np.zeros((2, NT), dtype=np.int64)
    for h in range(2):
        for t, (j, k) in enumerate(halves[h]):
            tri_j[h, t], tri_k[h, t] = j, k
        for t2, j in enumerate(range(h * 16, (h + 1) * 16)):
            tri_j[h, NOFF + t2] = tri_k[h, NOFF + t2] = j
    return tri_j, tri_k


_TRI_J, _TRI_K = _pair_index()


def _build():
    nc = bacc.Bacc("TRN2", target_bir_lowering=False, debug=False)
    inp_d = nc.declare_dram_parameter("inp", [P, NIN], F32, isOutput=False)
    cst_d = nc.declare_dram_parameter("cst", [P, 64], F32, isOutput=False)
    out_d = nc.declare_dram_parameter("out", [B_LOC * N, L], F32, isOutput=True)

    TWO_PI = float(2.0 * np.pi)

    with tile.TileContext(nc) as tc, ExitStack() as ctx:
        pool = ctx.enter_context(tc.tile_pool(name="sb", bufs=1))
        rad_pool = ctx.enter_context(tc.tile_pool(name="rad", bufs=2))
        w_pool = ctx.enter_context(tc.tile_pool(name="w", bufs=2))
        scr_pool = ctx.enter_context(tc.tile_pool(name="scr", bufs=3))
        psum = ctx.enter_context(tc.tile_pool(name="ps", bufs=1, space="PSUM"))

        def big(tag, dt=F32):
            return pool.tile([P, NT], dt, name=tag, tag=tag)

        # split input DMAs so consumers start as soon as their slice lands
        cj = pool.tile([P, 3, NT], F32, name="cj", tag="cj")
        ck = pool.tile([P, 3, NT], F32, name="ck", tag="ck")
        ci = pool.tile([P, 3], F32, name="ci", tag="ci")
        uf = pool.tile([P, 4, NT], F32, name="uf", tag="uf")
        cst = pool.tile([P, 64], F32, name="cst", tag="cst")
        nc.sync.dma_start(ci[:], inp_d[:, OFF_CI : OFF_CI + 3])
        nc.sync.dma_start(uf[:], inp_d[:, OFF_UJ : OFF_UJ + 4 * NT].rearrange("p (c t) -> p c t", c=4))
        nc.scalar.dma_start(cj[:], inp_d[:, OFF_CJ : OFF_CJ + 3 * NT].rearrange("p (c t) -> p c t", c=3))
        nc.gpsimd.dma_start(ck[:], inp_d[:, OFF_CK : OFF_CK + 3 * NT].rearrange("p (c t) -> p c t", c=3))
        nc.gpsimd.dma_start(cst[:], cst_d[:])
        uj, uk, fj, fk = uf[:, 0, :], uf[:, 1, :], uf[:, 2, :], uf[:, 3, :]

        # ---- vj = xj - xi (= -v_j), vk = xk - xi (= -v_k); dot = vj.vk ----
        vj = pool.tile([P, 3, NT], F32, name="vj", tag="vj")
        vk = pool.tile([P, 3, NT], F32, name="vk", tag="vk")
        for c in range(3):
            nc.vector.tensor_scalar(vj[:, c, :], cj[:, c, :], ci[:, c : c + 1], None, OP.subtract)
            nc.vector.tensor_scalar(vk[:, c, :], ck[:, c, :], ci[:, c : c + 1], None, OP.subtract)

        prod = big("prod")
        dot = big("dot")
        nc.vector.tensor_tensor(dot[:], vj[:, 0, :], vk[:, 0, :], OP.mult)
        for c in (1, 2):
            nc.vector.tensor_tensor(prod[:], vj[:, c, :], vk[:, c, :], OP.mult)
            nc.vector.tensor_tensor(dot[:], dot[:], prod[:], OP.add)

        # ---- thp = theta / 2pi  (theta = dot / (uj*uk + 1e-5)) ----
        den = big("den")
        nc.gpsimd.tensor_tensor(den[:], uj, uk, OP.mult)
        nc.vector.tensor_scalar(den[:], den[:], 1e-5, TWO_PI, OP.add, OP.mult)
        rden = big("rden")
        nc.vector.reciprocal_approx_fast(rden[:], den[:])
        thp = big("thp")
        nc.vector.tensor_tensor(thp[:], dot[:], rden[:], OP.mult)

        # ---- radial stage (emitted before trig so ACT groups Exp with Square:
        #      exp_and_friends loads once, then trig_and_small once) ----
        q = big("q")
        nc.gpsimd.tensor_tensor(q[:], uj, uk, OP.add)
        cut = big("cut")
        nc.vector.scalar_tensor_tensor(
            cut[:, :NOFF], fj[:, :NOFF], 0.25, fk[:, :NOFF], OP.mult, OP.mult)
        nc.vector.scalar_tensor_tensor(
            cut[:, NOFF:], fj[:, NOFF:], 0.125, fk[:, NOFF:], OP.mult, OP.mult)
        rs_bias = pool.tile([P, NR], F32, name="rs_bias", tag="rs_bias")
        for r in range(NR):
            nc.vector.memset(rs_bias[:, r : r + 1], float(-RS_VALS[r]))
        Ws = []
        exp_insts = []
        for r in range(NR):
            sq = rad_pool.tile([P, NT], F32, name=f"sq{r}", tag="sq")
            nc.scalar.activation(sq[:], q[:], ACT.Square, bias=rs_bias[:, r : r + 1], scale=0.5)
            rad = rad_pool.tile([P, NT], F32, name=f"rad{r}", tag="rad")
            exp_insts.append(nc.scalar.activation(rad[:], sq[:], ACT.Exp, scale=float(-ITA)))
            W = w_pool.tile([P, NT], F32, name=f"w{r}", tag=f"w{r}")
            nc.gpsimd.tensor_tensor(W[:], cut[:], rad[:], OP.mult)
            Ws.append(W)

        # ---- range-reduce + sin/cos via Sin table ----
        # round(r) for |r| < 2^22 via the f32 magic constant: (r + 1.5*2^23) - 1.5*2^23
        RC = float(12582912.0)

        def trig(src, out_tag):
            n_f = big(out_tag + "_nf")
            nc.vector.tensor_scalar(n_f[:], src[:], RC, RC, OP.add, OP.subtract)
            fr = big(out_tag + "_fr")
            nc.gpsimd.tensor_tensor(fr[:], src[:], n_f[:], OP.subtract)
            o = big(out_tag)
            sin_inst = nc.scalar.activation(o[:], fr[:], ACT.Sin, scale=TWO_PI)
            return o, sin_inst

        s1, sin1_inst = trig(thp, "s1")
        r2 = big("r2")
        nc.vector.tensor_scalar(r2[:], thp[:], 0.25, None, OP.add)
        c1, sin2_inst = trig(r2, "c1")

        from concourse.tile import add_dep_helper
        for ei in exp_insts:
            add_dep_helper(ei.ins, sin1_inst.ins, sync=False, reason="group exp-set before trig-set")
            add_dep_helper(ei.ins, sin2_inst.ins, sync=False, reason="group exp-set before trig-set")

        # ---- 4 angular fields (1+-c)^4, (1+-s)^4 via two chained Squares on ScalarE ----
        bias_one = pool.tile([P, 1], F32, name="bias_one", tag="bias_one")
        nc.vector.memset(bias_one[:], 1.0)
        angs = []
        for nm, src, sc in (("bp", s1, 1.0), ("bm", s1, -1.0), ("ap", c1, 1.0), ("am", c1, -1.0)):
            g = big("g_" + nm)
            nc.scalar.activation(g[:], src[:], ACT.Square, bias=bias_one[:], scale=sc)
            a = big("ang_" + nm)
            nc.scalar.activation(a[:], g[:], ACT.Square)
            angs.append(a)


        spart = pool.tile([P, 24], F32, name="spart", tag="spart")

        # ---- fused reduces sum_t W_r * ang_m ; angs order (bp,bm,ap,am) -> m col (1,3,0,2)
        for mi, mcol in ((0, 1), (1, 3), (2, 0), (3, 2)):
            for r in range(NR):
                scr = scr_pool.tile([P, NT], F32, name=f"scr{r}{mcol}", tag="scr")
                nc.vector.scalar_tensor_tensor(
                    scr[:], Ws[r][:], 0.0, angs[mi][:], OP.bypass, OP.mult,
                    accum_out=spart[:, r * NM + mcol : r * NM + mcol + 1])

        # ---- combine: pair-sum over half partitions; assemble all 40 l-columns in PSUM
        #      l = lam*20 + r*4 + t ; lam=+1 -> m=t ; lam=-1 -> m=(t+2)%4 ----
        s2p = psum.tile([64, L], F32, name="s2p", tag="s2p")
        sp3 = spart[:, 0 : NR * NM].rearrange("p (r t) -> p r t", r=NR, t=NM)
        nc.tensor.matmul(s2p[:, 0:20], cst[:, 0:64], spart[:, 0 : NR * NM])
        o3 = s2p[:].rearrange("n (g r t) -> n g r t", g=2, r=NR, t=NM)
        nc.tensor.matmul(o3[:, 1, :, 0:2], cst[:, 0:64], sp3[:, :, 2:4])
        nc.tensor.matmul(o3[:, 1, :, 2:4], cst[:, 0:64], sp3[:, :, 0:2])
        s2s = pool.tile([64, L], F32, name="s2s", tag="s2s")
        nc.vector.tensor_copy(s2s[:], s2p[:])
        nc.sync.dma_start(out_d[:], s2s[:])

    nc.compile()
    return nc


def _ensure_ntff_hook():
    """Register the axon NTFF profiling hook if the image lacks antenv.axon_hooks."""
    import types

    try:
        from antenv.axon_hooks import get_axon_ntff_profile_hook
        if get_axon_ntff_profile_hook() is not None:
            return
        have_mod = True
    except ImportError:
        have_mod = False
    try:
        if "/root/.axon_site" not in sys.path:
            sys.path.insert(0, "/root/.axon_site")
        from trn_agent_boot.trn_boot import _ntff_profile_via_ctypes

        hook = _ntff_profile_via_ctypes("/opt/axon/libaxon_pjrt.so")
        if hook is None:
            return
    except Exception:
        return
    if have_mod:
        from antenv import axon_hooks
        axon_hooks.set_axon_ntff_profile_hook(hook)
    else:
        m = types.ModuleType("antenv.axon_hooks")
        _h = [hook]
        m.get_axon_ntff_profile_hook = lambda: _h[0]
        m.set_axon_ntff_profile_hook = lambda h: _h.__setitem__(0, h)
        import antenv
        antenv.axon_hooks = m
        sys.modules["antenv.axon_hooks"] = m


_NC = None


def _get_nc():
    global _NC
    if _NC is None:
        _NC = _build()
    return _NC


def _host_pack(d_cutoff, d, atom_coordinates):
    """Pure gather/replication of raw inputs into the per-core packed layout."""
    d_cutoff = np.ascontiguousarray(d_cutoff, dtype=np.float32)
    d = np.ascontiguousarray(d, dtype=np.float32)
    coords = np.ascontiguousarray(atom_coordinates, dtype=np.float32)

    p = np.arange(P)
    b_of_p = p // (N * 2)          # [P]
    i_of_p = (p // 2) % N          # [P]
    half = p % 2                   # [P]
    jt = _TRI_J[half]              # [P, NT]
    kt = _TRI_K[half]              # [P, NT]

    in_maps = []
    for c in range(NCORES):
        cd = coords[c * B_LOC : (c + 1) * B_LOC]
        dd = d[c * B_LOC : (c + 1) * B_LOC]
        fc = d_cutoff[c * B_LOC : (c + 1) * B_LOC]
        buf = np.empty((P, NIN), dtype=np.float32)
        buf[:, OFF_CI : OFF_CI + 3] = cd[b_of_p, i_of_p]
        buf[:, OFF_CJ : OFF_CJ + 3 * NT] = (
            cd[b_of_p[:, None], jt].transpose(0, 2, 1).reshape(P, 3 * NT))
        buf[:, OFF_CK : OFF_CK + 3 * NT] = (
            cd[b_of_p[:, None], kt].transpose(0, 2, 1).reshape(P, 3 * NT))
        buf[:, OFF_UJ : OFF_UJ + NT] = dd[b_of_p[:, None], i_of_p[:, None], jt]
        buf[:, OFF_UK : OFF_UK + NT] = dd[b_of_p[:, None], i_of_p[:, None], kt]
        buf[:, OFF_FJ : OFF_FJ + NT] = fc[b_of_p[:, None], i_of_p[:, None], jt]
        buf[:, OFF_FK : OFF_FK + NT] = fc[b_of_p[:, None], i_of_p[:, None], kt]
        in_maps.append({"inp": buf, "cst": _const_blob()})
    return in_maps


_CST = None


def _const_blob():
    global _CST
    if _CST is None:
        cst = np.zeros((P, 64), dtype=np.float32)
        cst[:, 0:64] = np.repeat(np.eye(64, dtype=np.float32), 2, axis=0)
        _CST = cst
    return _CST


def kernel(d_cutoff, d, atom_coordinates, _trace=False):
    if _trace:
        _ensure_ntff_hook()
    nc = _get_nc()
    in_maps = _host_pack(d_cutoff, d, atom_coordinates)
    res = run_bass_kernel_spmd(nc, in_maps, core_ids=list(range(NCORES)), trace=_trace)
    out = np.concatenate(
        [res.results[c]["out"].reshape(B_LOC, N, L) for c in range(NCORES)], axis=0
    ).astype(np.float32)
    if _trace:
        kernel._last_results = res
    return out


# revision 16
# speedup vs baseline: 1.7191x; 1.0591x over previous
"""Trainium2 Bass kernel for AngularSymmetryMod (ANI-style angular symmetry functions).

Math: out[b,i,l] = sum_{j,k} (1+lam*cos(theta-theta_t))^zeta * exp(-ita*((R_ij+R_ik)/2-Rs)^2)
                            * f_ij*f_ik * 2^(1-zeta)
over a 40-point parameter grid l=(lam in {+-1}, 5 Rs values, 4 theta_t values), zeta=4.

Key optimizations:
 1. theta_t = {0.0, 1.57, 3.14, 4.71} are (to 8e-4) the exact quadrants {0, pi/2, pi, 3pi/2},
    so cos(theta-theta_t) = {c, s, -c, -s} and the angular factor collapses to FOUR distinct
    fields: (1+-c)^4, (1+-s)^4 — each two chained Square activations on the ScalarEngine.
    (Validated: 2.2e-4 rel err vs the f32 reference.)
 2. Each of the 40 outputs is S[r, m] (5 radials x 4 angulars = 20 reductions); the 40 outputs
    are a column remap handled by the output DMA.
 3. The (j,k) summand is symmetric, so only the 528 pairs j<=k are computed (host gathers the
    packed pair layout; off-diagonal weight 2 is folded into the cutoff product on-chip).
 4. sin/cos need exact-range reduction (theta spans +-2.3e6): theta/2pi - round(theta/2pi) via
    the DVE's round-to-nearest f32->int32 convert, then the Sin table on [-pi, pi].

Sharding: data-parallel over batch (16 molecules -> 2 per core on 8 cores). No collectives.
Layout per core: 128 partitions = (b_loc:2, i:32, half:2), free = 264 packed (j,k) pairs
(248 off-diagonal + 16 diagonal per half).
"""

import sys
import numpy as np

sys.path.insert(0, "/opt/trn_rl_repo")

from contextlib import ExitStack

import concourse.bass as bass
import concourse.tile as tile
from concourse import bacc, mybir
from concourse.bass_utils import run_bass_kernel_spmd

B, N, L = 16, 32, 40
NCORES = 8
B_LOC = B // NCORES  # 2
P = 128  # partitions = B_LOC * N * 2
NT = 264           # packed pairs per partition-half
NOFF = 248         # off-diagonal entries (first NOFF of NT); rest are diagonal

BOHR = 0.52917721092
ITA = 1.12
ZETA = 4.0
RS_VALS = np.array([0.5, 1.17, 1.83, 2.5, 3.17]) / BOHR
NR, NM = 5, 4

F32 = mybir.dt.float32
I32 = mybir.dt.int32
OP = mybir.AluOpType
ACT = mybir.ActivationFunctionType

# free-axis offsets in the single packed [128, NIN] input tensor
OFF_CI = 0                 # [3]       coords of atom i (per-partition scalars)
OFF_CJ = 3                 # [3*NT]    coords of j-atom of pair t, layout [c][t]
OFF_CK = 3 + 3 * NT        # [3*NT]    coords of k-atom of pair t
OFF_UJ = 3 + 6 * NT        # [NT]      d[b,i,j_t]
OFF_UK = OFF_UJ + NT       # [NT]      d[b,i,k_t]
OFF_FJ = OFF_UK + NT       # [NT]      d_cutoff[b,i,j_t]
OFF_FK = OFF_FJ + NT       # [NT]      d_cutoff[b,i,k_t]
NIN = 3 + 10 * NT


def _pair_index():
    """Static (j,k) pair enumeration: per half, 248 off-diagonal + 16 diagonal."""
    pairs = [(j, k) for j in range(N) for k in range(j + 1, N)]  # 496
    halves = [pairs[0::2], pairs[1::2]]
    tri_j = np.zeros((2, NT), dtype=np.int64)
    tri_k = np.zeros((2, NT), dtype=np.int64)
    for h in range(2):
        for t, (j, k) in enumerate(halves[h]):
            tri_j[h, t], tri_k[h, t] = j, k
        for t2, j in enumerate(range(h * 16, (h + 1) * 16)):
            tri_j[h, NOFF + t2] = tri_k[h, NOFF + t2] = j
    return tri_j, tri_k


_TRI_J, _TRI_K = _pair_index()


def _build():
    nc = bacc.Bacc("TRN2", target_bir_lowering=False, debug=False)
    inp_d = nc.declare_dram_parameter("inp", [P, NIN], F32, isOutput=False)
    cst_d = nc.declare_dram_parameter("cst", [P, 64], F32, isOutput=False)
    out_d = nc.declare_dram_parameter("out", [B_LOC * N, L], F32, isOutput=True)

    TWO_PI = float(2.0 * np.pi)

    with tile.TileContext(nc) as tc, ExitStack() as ctx:
        pool = ctx.enter_context(tc.tile_pool(name="sb", bufs=1))
        rad_pool = ctx.enter_context(tc.tile_pool(name="rad", bufs=2))
        w_pool = ctx.enter_context(tc.tile_pool(name="w", bufs=2))
        scr_pool = ctx.enter_context(tc.tile_pool(name="scr", bufs=3))
        psum = ctx.enter_context(tc.tile_pool(name="ps", bufs=1, space="PSUM"))

        def big(tag, dt=F32):
            return pool.tile([P, NT], dt, name=tag, tag=tag)

        # split input DMAs so consumers start as soon as their slice lands
        cj = pool.tile([P, 3, NT], F32, name="cj", tag="cj")
        ck = pool.tile([P, 3, NT], F32, name="ck", tag="ck")
        ci = pool.tile([P, 3], F32, name="ci", tag="ci")
        uf = pool.tile([P, 4, NT], F32, name="uf", tag="uf")
        cst = pool.tile([P, 64], F32, name="cst", tag="cst")
        nc.sync.dma_start(ci[:], inp_d[:, OFF_CI : OFF_CI + 3])
        nc.sync.dma_start(cj[:], inp_d[:, OFF_CJ : OFF_CJ + 3 * NT].rearrange("p (c t) -> p c t", c=3))
        nc.sync.dma_start(ck[:], inp_d[:, OFF_CK : OFF_CK + 3 * NT].rearrange("p (c t) -> p c t", c=3))
        nc.gpsimd.dma_start(uf[:], inp_d[:, OFF_UJ : OFF_UJ + 4 * NT].rearrange("p (c t) -> p c t", c=4))
        nc.gpsimd.dma_start(cst[:], cst_d[:])
        uj, uk, fj, fk = uf[:, 0, :], uf[:, 1, :], uf[:, 2, :], uf[:, 3, :]

        # ---- vj = xj - xi (= -v_j), vk = xk - xi (= -v_k); dot = vj.vk ----
        vj = pool.tile([P, 3, NT], F32, name="vj", tag="vj")
        vk = pool.tile([P, 3, NT], F32, name="vk", tag="vk")
        for c in range(3):
            nc.vector.tensor_scalar(vj[:, c, :], cj[:, c, :], ci[:, c : c + 1], None, OP.subtract)
            nc.vector.tensor_scalar(vk[:, c, :], ck[:, c, :], ci[:, c : c + 1], None, OP.subtract)

        prod = big("prod")
        dot = big("dot")
        nc.vector.tensor_tensor(dot[:], vj[:, 0, :], vk[:, 0, :], OP.mult)
        for c in (1, 2):
            nc.vector.tensor_tensor(prod[:], vj[:, c, :], vk[:, c, :], OP.mult)
            nc.vector.tensor_tensor(dot[:], dot[:], prod[:], OP.add)

        # ---- thp = theta / 2pi  (theta = dot / (uj*uk + 1e-5)) ----
        den = big("den")
        nc.gpsimd.tensor_tensor(den[:], uj, uk, OP.mult)
        nc.vector.tensor_scalar(den[:], den[:], 1e-5, TWO_PI, OP.add, OP.mult)
        rden = big("rden")
        nc.vector.reciprocal_approx_fast(rden[:], den[:])
        thp = big("thp")
        nc.vector.tensor_tensor(thp[:], dot[:], rden[:], OP.mult)

        # ---- radial stage (emitted before trig so ACT groups Exp with Square:
        #      exp_and_friends loads once, then trig_and_small once) ----
        q = big("q")
        nc.gpsimd.tensor_tensor(q[:], uj, uk, OP.add)
        cut = big("cut")
        nc.vector.scalar_tensor_tensor(
            cut[:, :NOFF], fj[:, :NOFF], 0.25, fk[:, :NOFF], OP.mult, OP.mult)
        nc.vector.scalar_tensor_tensor(
            cut[:, NOFF:], fj[:, NOFF:], 0.125, fk[:, NOFF:], OP.mult, OP.mult)
        rs_bias = pool.tile([P, NR], F32, name="rs_bias", tag="rs_bias")
        for r in range(NR):
            nc.vector.memset(rs_bias[:, r : r + 1], float(-RS_VALS[r]))
        Ws = []
        exp_insts = []
        for r in range(NR):
            sq = rad_pool.tile([P, NT], F32, name=f"sq{r}", tag="sq")
            nc.scalar.activation(sq[:], q[:], ACT.Square, bias=rs_bias[:, r : r + 1], scale=0.5)
            rad = rad_pool.tile([P, NT], F32, name=f"rad{r}", tag="rad")
            exp_insts.append(nc.scalar.activation(rad[:], sq[:], ACT.Exp, scale=float(-ITA)))
            W = w_pool.tile([P, NT], F32, name=f"w{r}", tag=f"w{r}")
            nc.gpsimd.tensor_tensor(W[:], cut[:], rad[:], OP.mult)
            Ws.append(W)

        # ---- range-reduce + sin/cos via Sin table ----
        # round(r) for |r| < 2^22 via the f32 magic constant: (r + 1.5*2^23) - 1.5*2^23
        RC = float(12582912.0)

        def trig(src, out_tag):
            n_f = big(out_tag + "_nf")
            nc.vector.tensor_scalar(n_f[:], src[:], RC, RC, OP.add, OP.subtract)
            fr = big(out_tag + "_fr")
            nc.gpsimd.tensor_tensor(fr[:], src[:], n_f[:], OP.subtract)
            o = big(out_tag)
            sin_inst = nc.scalar.activation(o[:], fr[:], ACT.Sin, scale=TWO_PI)
            return o, sin_inst

        s1, sin1_inst = trig(thp, "s1")
        r2 = big("r2")
        nc.vector.tensor_scalar(r2[:], thp[:], 0.25, None, OP.add)
        c1, sin2_inst = trig(r2, "c1")

        from concourse.tile import add_dep_helper
        for ei in exp_insts:
            add_dep_helper(sin1_inst.ins, ei.ins, sync=False, reason="group exp-set before trig-set")
            add_dep_helper(sin2_inst.ins, ei.ins, sync=False, reason="group exp-set before trig-set")

        # ---- 4 angular fields (1+-c)^4, (1+-s)^4 via two chained Squares on ScalarE ----
        bias_one = pool.tile([P, 1], F32, name="bias_one", tag="bias_one")
        nc.vector.memset(bias_one[:], 1.0)
        angs = []
        for nm, src, sc in (("bp", s1, 1.0), ("bm", s1, -1.0), ("ap", c1, 1.0), ("am", c1, -1.0)):
            g = big("g_" + nm)
            nc.scalar.activation(g[:], src[:], ACT.Square, bias=bias_one[:], scale=sc)
            a = big("ang_" + nm)
            nc.scalar.activation(a[:], g[:], ACT.Square)
            angs.append(a)


        spart = pool.tile([P, 24], F32, name="spart", tag="spart")

        # ---- fused reduces sum_t W_r * ang_m ; angs order (bp,bm,ap,am) -> m col (1,3,0,2)
        for mi, mcol in ((0, 1), (1, 3), (2, 0), (3, 2)):
            for r in range(NR):
                scr = scr_pool.tile([P, NT], F32, name=f"scr{r}{mcol}", tag="scr")
                nc.vector.scalar_tensor_tensor(
                    scr[:], Ws[r][:], 0.0, angs[mi][:], OP.bypass, OP.mult,
                    accum_out=spart[:, r * NM + mcol : r * NM + mcol + 1])

        # ---- combine: pair-sum over half partitions; assemble all 40 l-columns in PSUM
        #      l = lam*20 + r*4 + t ; lam=+1 -> m=t ; lam=-1 -> m=(t+2)%4 ----
        s2p = psum.tile([64, L], F32, name="s2p", tag="s2p")
        sp3 = spart[:, 0 : NR * NM].rearrange("p (r t) -> p r t", r=NR, t=NM)
        nc.tensor.matmul(s2p[:, 0:20], cst[:, 0:64], spart[:, 0 : NR * NM])
        o3 = s2p[:].rearrange("n (g r t) -> n g r t", g=2, r=NR, t=NM)
        nc.tensor.matmul(o3[:, 1, :, 0:2], cst[:, 0:64], sp3[:, :, 2:4])
        nc.tensor.matmul(o3[:, 1, :, 2:4], cst[:, 0:64], sp3[:, :, 0:2])
        s2s = pool.tile([64, L], F32, name="s2s", tag="s2s")
        nc.vector.tensor_copy(s2s[:], s2p[:])
        nc.sync.dma_start(out_d[:], s2s[:])

    nc.compile()
    return nc


def _ensure_ntff_hook():
    """Register the axon NTFF profiling hook if the image lacks antenv.axon_hooks."""
    import types

    try:
        from antenv.axon_hooks import get_axon_ntff_profile_hook
        if get_axon_ntff_profile_hook() is not None:
            return
        have_mod = True
    except ImportError:
        have_mod = False
    try:
        if "/root/.axon_site" not in sys.path:
            sys.path.insert(0, "/root/.axon_site")
        from trn_agent_boot.trn_boot import _ntff_profile_via_ctypes

        hook = _ntff_profile_via_ctypes("/opt/axon/libaxon_pjrt.so")
        if hook is None:
            return
    except Exception:
        return
    if have_mod:
        from antenv import axon_hooks
        axon_hooks.set_axon_ntff_profile_hook(hook)
    else:
        m = types.ModuleType("antenv.axon_hooks")
        _h = [hook]
        m.get_axon_ntff_profile_hook = lambda: _h[0]
        m.set_axon_ntff_profile_hook = lambda h: _h.__setitem__(0, h)
        import antenv
        antenv.axon_hooks = m
        sys.modules["antenv.axon_hooks"] = m


_NC = None


def _get_nc():
    global _NC
    if _NC is None:
        _NC = _build()
    return _NC


def _host_pack(d_cutoff, d, atom_coordinates):
    """Pure gather/replication of raw inputs into the per-core packed layout."""
    d_cutoff = np.ascontiguousarray(d_cutoff, dtype=np.float32)
    d = np.ascontiguousarray(d, dtype=np.float32)
    coords = np.ascontiguousarray(atom_coordinates, dtype=np.float32)

    p = np.arange(P)
    b_of_p = p // (N * 2)          # [P]
    i_of_p = (p // 2) % N          # [P]
    half = p % 2                   # [P]
    jt = _TRI_J[half]              # [P, NT]
    kt = _TRI_K[half]              # [P, NT]

    in_maps = []
    for c in range(NCORES):
        cd = coords[c * B_LOC : (c + 1) * B_LOC]
        dd = d[c * B_LOC : (c + 1) * B_LOC]
        fc = d_cutoff[c * B_LOC : (c + 1) * B_LOC]
        buf = np.empty((P, NIN), dtype=np.float32)
        buf[:, OFF_CI : OFF_CI + 3] = cd[b_of_p, i_of_p]
        buf[:, OFF_CJ : OFF_CJ + 3 * NT] = (
            cd[b_of_p[:, None], jt].transpose(0, 2, 1).reshape(P, 3 * NT))
        buf[:, OFF_CK : OFF_CK + 3 * NT] = (
            cd[b_of_p[:, None], kt].transpose(0, 2, 1).reshape(P, 3 * NT))
        buf[:, OFF_UJ : OFF_UJ + NT] = dd[b_of_p[:, None], i_of_p[:, None], jt]
        buf[:, OFF_UK : OFF_UK + NT] = dd[b_of_p[:, None], i_of_p[:, None], kt]
        buf[:, OFF_FJ : OFF_FJ + NT] = fc[b_of_p[:, None], i_of_p[:, None], jt]
        buf[:, OFF_FK : OFF_FK + NT] = fc[b_of_p[:, None], i_of_p[:, None], kt]
        in_maps.append({"inp": buf, "cst": _const_blob()})
    return in_maps


_CST = None


def _const_blob():
    global _CST
    if _CST is None:
        cst = np.zeros((P, 64), dtype=np.float32)
        cst[:, 0:64] = np.repeat(np.eye(64, dtype=np.float32), 2, axis=0)
        _CST = cst
    return _CST


def kernel(d_cutoff, d, atom_coordinates, _trace=False):
    if _trace:
        _ensure_ntff_hook()
    nc = _get_nc()
    in_maps = _host_pack(d_cutoff, d, atom_coordinates)
    res = run_bass_kernel_spmd(nc, in_maps, core_ids=list(range(NCORES)), trace=_trace)
    out = np.concatenate(
        [res.results[c]["out"].reshape(B_LOC, N, L) for c in range(NCORES)], axis=0
    ).astype(np.float32)
    if _trace:
        kernel._last_results = res
    return out
